# revision 14
# baseline (speedup 1.0000x reference)
"""Banded (sliding-window) multi-head attention for TRN2, 8 NeuronCores.

Problem: nn_BaseAttention (B=2, T=4096, C=512, H=8, hd=64, WIN=128).
  qkv = x @ W_qkv ; banded softmax(q k^T / sqrt(hd), |i-j|<=WIN) @ v ; @ W_out + b_out

Sharding: 8 cores = 2 batches x 4 T-chunks of 1024 queries. Each core gets its
x rows plus a 128-row halo on each side (zero-padded at sequence edges) and
full replicated weights; it produces its own [1024, 512] output slice, so the
host-side gather is pure concatenation (no cross-core reduction).

Device pipeline per core (all layouts chosen to avoid transposing activations):
  xT   = x^T direct DMA (host pre-transposes)          [C, 1280]
  q^T/k^T = W_qkv-slice-stationary matmuls             [hd, rows]  (head-major)
  v    = natural matmuls (lhsT = xT)                   [rows, hd] fused [V|1]
  S^T  = k^T-stationary matmuls -> one contiguous      [128 keys*, 1536 qcols]
         3-bank PSUM strip per head (8 piece matmuls, bank-aligned)
  exp  = ONE activation per head strip (PSUM->SBUF f16)
  mask = ONE elementwise mul with a host-precomputed band+edge mask
  O^T|sums = [V|1]-stationary matmuls, split-region PSUM opening so only
         true-band columns are streamed (1536 cols vs 2048 padded)
  rs   = reciprocal_approx_fast(sums) straight from PSUM
  O^T * rs -> oa ; Y = oa-stationary @ W_out + b_out
"""

import numpy as np
from contextlib import ExitStack

import concourse.bass as bass
from concourse import bacc
import concourse.mybir as mybir
import concourse.tile as tile
from concourse.bass_utils import run_bass_kernel_spmd

# ----- problem constants (hardcoded per contest contract) -----
B, T, C = 2, 4096, 512
H, HD, WIN = 8, 64, 128
NCORES = 8
CHUNK = 1024                # queries per core
ROWS = CHUNK + 2 * WIN      # 1280 rows incl. halo
QCW = 512                   # query-chunk width
NQC = CHUNK // QCW          # 2
NKT = 6                     # key tiles per query-chunk
ESW = 1536                  # true band width per (qc, head): sum of kt widths
SCALE = HD ** -0.5

F32 = mybir.dt.float32
F16 = mybir.dt.float16
BF16 = mybir.dt.bfloat16
EXP = mybir.ActivationFunctionType.Exp

# S^T piece table: (kt, cum0, cum1, q0, start, stop).  The [128, 1536] PSUM
# strip spans 3 banks; piece matmuls are split at bank boundaries (512-col
# granularity in f32).  q0 is the first query col of the piece; within kt,
# q = tc0(kt) + (cum - cumstart(kt)).
_S_PIECES = [
    (0,    0,  128,   0, True,  False),
    (1,  128,  384,   0, False, False),
    (2,  384,  512,   0, False, True),
    (2,  512,  768, 128, True,  False),
    (3,  768, 1024, 128, False, True),
    (3, 1024, 1152, 384, True,  False),
    (4, 1152, 1408, 256, False, False),
    (5, 1408, 1536, 384, False, True),
]

# AV piece table: (kt, es_c0, es_c1, out_q0, out_q1, start, stop).  Split so
# every matmul's output region is uniformly fresh or uniformly accumulating
# (PSUM pending-zero semantics), with one start (whole-bank clear) and one
# stop.  Total moving width 1536 = the true band, no padding.
_AV_PIECES = [
    (0,    0,  128,   0, 128, True,  False),
    (1,  256,  384, 128, 256, False, False),
    (1,  128,  256,   0, 128, False, False),
    (2,  640,  768, 256, 384, False, False),
    (2,  384,  640,   0, 256, False, False),
    (3, 1024, 1152, 384, 512, False, False),
    (3,  768, 1024, 128, 384, False, False),
    (4, 1152, 1408, 256, 512, False, False),
    (5, 1408, 1536, 384, 512, False, True),
]


def build_attention_body(tc, y, xh, wqkv, wout, bout, mask):
    """Emit the per-core kernel. All APs are DRAM tensors.

    y     [1024, 512] out     xh   [512, 1280] in (halo'd x rows, pre-T)
    wqkv  [512, 1536]  in (q-block pre-scaled by hd^-0.5 on host)
    wout  [512, 512]   in     bout [1, 512] in
    mask  [128, 3072]  in (f16 0/1 band+edge mask, one 1536 strip per qc)
    """
    nc = tc.nc

    with ExitStack() as ctx:
        sb = ctx.enter_context(tc.tile_pool(name="sb", bufs=1))

        # ---- constants / persistent tiles ----
        ones_f = sb.tile([128, 128], F32, tag="ones_f", name="ones_f")
        nc.gpsimd.memset(ones_f[:], 1.0)
        ones_b = sb.tile([128, 128], F16, tag="ones_b", name="ones_b")
        nc.vector.tensor_copy(ones_b[:], ones_f[:])

        xT = [sb.tile([128, ROWS], F16, tag=f"xT{i}", name=f"xT{i}") for i in range(4)]
        for ct in range(4):
            eng = nc.sync if ct % 2 == 0 else nc.scalar
            eng.dma_start(xT[ct][:], xh[128 * ct:128 * (ct + 1), :])

        bo = sb.tile([1, C], F16, tag="bo", name="bo")
        nc.gpsimd.dma_start(bo[:], bout[:])
        wq_sb = []
        for i in range(4):
            w_i = sb.tile([128, 3 * C], F16, tag=f"wq{i}", name=f"wq{i}")
            eng = nc.sync if i % 2 == 0 else nc.scalar
            eng.dma_start(w_i[:], wqkv[128 * i:128 * (i + 1), :])
            wq_sb.append(w_i)
        wo_sb = []
        for i in range(4):
            w_i = sb.tile([128, C], F16, tag=f"wo{i}", name=f"wo{i}")
            nc.gpsimd.dma_start(w_i[:], wout[128 * i:128 * (i + 1), :])
            wo_sb.append(w_i)
        mk = sb.tile([128, NQC * ESW], BF16, tag="mk", name="mk")
        nc.gpsimd.dma_start(mk[:, 0:ESW], mask[:, 0:ESW])
        nc.sync.dma_start(mk[:, ESW:2 * ESW], mask[:, ESW:2 * ESW])

        # preload the exp activation-table set during the DMA prologue so the
        # ~2.7us table load is off the attention critical path
        scr = sb.tile([128, 1], F32, tag="scr", name="scr")
        nc.scalar.activation(scr[:], ones_f[:, 0:1], EXP)

        qT = [sb.tile([128, CHUNK], F16, tag=f"qT{i}", name=f"qT{i}") for i in range(4)]
        kT = [sb.tile([128, ROWS], F16, tag=f"kT{i}", name=f"kT{i}") for i in range(4)]
        # fused V|ones stationary tiles: vp0 blocks = [V_h | 1] for even h,
        # vp1 blocks = [1 | V_h] for odd h (ones column folds the softmax
        # denominator into the AV matmul)
        vp0 = [sb.tile([128, C], BF16, tag=f"vp0_{i}", name=f"vp0_{i}") for i in range(10)]
        vp1 = [sb.tile([128, C], BF16, tag=f"vp1_{i}", name=f"vp1_{i}") for i in range(10)]
        for i in range(10):
            o0 = vp0[i][:].rearrange("p (b t c) -> p b t c", t=2, c=HD)
            o1 = vp1[i][:].rearrange("p (b t c) -> p b t c", t=2, c=HD)
            nc.gpsimd.memset(o0[:, :, 1, :], 1.0)
            nc.gpsimd.memset(o1[:, :, 0, :], 1.0)

        # ---- phase A+B: warmup + projections (own PSUM pool, freed after) ----
        with tc.tile_pool(name="pp", bufs=1, space="PSUM") as pp:
            # PE warm-up: ~3.5us of dummy matmuls during the DMA prologue so
            # the HAM clock gate reaches 8/8 before the real matmuls arrive.
            warm = pp.tile([128, 128], F32, tag="gp", bufs=4, name="warm")
            for _ in range(32):
                nc.tensor.matmul(warm[:], ones_f[:], ones_f[:], start=True, stop=True)

            # q^T / k^T: out[feat, rows]; lhsT = W_qkv block, rhs = xT
            for ft in range(8):
                if ft < 4:  # q feats, own rows only (local rows [128, 1152))
                    chunks = [(128, 512), (640, 512)]
                    dest, doff = qT[ft], -128
                else:       # k feats, all rows
                    chunks = [(0, 512), (512, 512), (1024, 256)]
                    dest, doff = kT[ft - 4], 0
                for r0, rw in chunks:
                    mm = pp.tile([128, QCW], F32, tag="gp", bufs=4, name="mmqk")
                    for ct in range(4):
                        nc.tensor.matmul(
                            mm[:, 0:rw],
                            wq_sb[ct][:, 128 * ft:128 * (ft + 1)],
                            xT[ct][:, r0:r0 + rw],
                            start=(ct == 0), stop=(ct == 3))
                    nc.any.tensor_copy(dest[:, r0 + doff:r0 + doff + rw], mm[:, 0:rw])
            # v natural: out[rows, vfeat]; lhsT = xT tile, rhs = W_qkv v-block
            for rt in range(10):
                mm = pp.tile([128, QCW], F32, tag="gp", bufs=4, name="mmv")
                for ct in range(4):
                    nc.tensor.matmul(
                        mm[:],
                        xT[ct][:, 128 * rt:128 * (rt + 1)],
                        wq_sb[ct][:, 1024:1536],
                        start=(ct == 0), stop=(ct == 3))
                m4 = mm[:].rearrange("p (b c) -> p b c", c=HD)
                d0 = vp0[rt][:].rearrange("p (b t c) -> p b t c", t=2, c=HD)
                d1 = vp1[rt][:].rearrange("p (b t c) -> p b t c", t=2, c=HD)
                nc.any.tensor_copy(d0[:, :, 0, :], m4[:, 0:4, :])
                nc.any.tensor_copy(d1[:, :, 1, :], m4[:, 4:8, :])

        # ---- phase C: banded attention ----
        # PSUM budget (16 KiB/partition, exact): 2 head-strips of 3 banks each
        # + 2 one-bank [O^T;sums] accumulators (tags av0/av1, shared with the
        # output-projection tiles).
        with tc.tile_pool(name="pa", bufs=1, space="PSUM") as pa:
            oall = [[None] * 4 for _ in range(NQC)]

            def emit_yproj(qc):
                for rb in range(4):
                    yp = pa.tile([128, C], F32, tag=f"av{rb % 2}", bufs=1, name="yp")
                    for pr in range(4):
                        nc.tensor.matmul(
                            yp[:],
                            oall[qc][pr][:, 128 * rb:128 * (rb + 1)],
                            wo_sb[pr][:],
                            start=(pr == 0), stop=False)
                    nc.tensor.matmul(yp[:], ones_b[0:1, :], bo[:],
                                     start=False, stop=True)
                    ys = sb.tile([128, C], F32, tag="ys", bufs=3, name="ys")
                    nc.any.tensor_copy(ys[:], yp[:])
                    r0 = 512 * qc + 128 * rb
                    nc.sync.dma_start(y[r0:r0 + 128, :], ys[:])

            for it in range(NQC * 4):
                qc, pr = divmod(it, 4)
                # --- S^T: 8 piece matmuls per head into a 3-bank strip ---
                esp = [pa.tile([128, ESW], F32, tag=f"esp{j}", bufs=1,
                               name=f"esp{j}") for j in range(2)]
                for kt, c0, c1, q0, st, sp_ in _S_PIECES:
                    kcol = 512 * qc + 128 * kt
                    w = c1 - c0
                    for j in range(2):
                        p0 = 64 * j
                        nc.tensor.matmul(
                            esp[j][:, c0:c1],
                            kT[pr][p0:p0 + 64, kcol:kcol + 128],
                            qT[pr][p0:p0 + 64,
                                   512 * qc + q0:512 * qc + q0 + w],
                            start=st, stop=sp_)
                # --- exp (one ACT per head strip) + band/edge mask mul ---
                es = []
                ers = []
                for j in range(2):
                    e_r = sb.tile([128, ESW], BF16, tag=f"er{j}", bufs=2,
                                  name=f"er{j}")
                    nc.scalar.activation(e_r[:], esp[j][:], EXP)
                    e_m = sb.tile([128, ESW], BF16, tag=f"em{j}", bufs=2,
                                  name=f"em{j}")
                    ers.append(e_r)
                    es.append(e_m)
                nc.gpsimd.tensor_mul(es[0][:, 0:1024], ers[0][:, 0:1024],
                                     mk[:, ESW * qc:ESW * qc + 1024])
                nc.vector.tensor_mul(es[0][:, 1024:ESW], ers[0][:, 1024:ESW],
                                     mk[:, ESW * qc + 1024:ESW * (qc + 1)])
                nc.vector.tensor_mul(es[1][:], ers[1][:], mk[:, ESW * qc:ESW * (qc + 1)])
                # --- fused [O^T ; sums] accumulation, true-band widths ---
                otp = [pa.tile([128, QCW], F32, tag=f"av{j}", bufs=1,
                               name=f"otp{j}") for j in range(2)]
                for j in range(2):
                    vp = vp0 if j == 0 else vp1
                    for kt, c0, c1, o0, o1, st, sp_ in _AV_PIECES:
                        nc.tensor.matmul(
                            otp[j][:, o0:o1],
                            vp[4 * qc + kt][:, 128 * pr:128 * pr + 128],
                            es[j][:, c0:c1],
                            start=st, stop=sp_)
                # --- normalize: rs = 1/sums, oa = O^T*rs.  The custom recip
                # op only runs at base partition 0, so each head needs one
                # partition-shifted copy (DVE/ACT support shifts; the custom
                # op does not).
                oa = sb.tile([128, QCW], F16, tag=f"oa{pr}", bufs=2, name=f"oa{pr}")
                rs = sb.tile([128, QCW], F32, tag="rs", bufs=2, name="rs")
                ss = sb.tile([128, 2 * QCW], F32, tag="ss", bufs=2, name="ss")
                # j0: sums live at rows 64-127 -> shift down, recip, aligned mul
                nc.scalar.copy(ss[0:64, 0:QCW], otp[0][64:128, :])
                nc.vector.reciprocal_approx_fast(rs[0:64, :], ss[0:64, 0:QCW])
                nc.vector.tensor_mul(oa[0:64, :], otp[0][0:64, :], rs[0:64, :])
                # j1: recip straight from PSUM at base 0, shift up, aligned mul
                nc.vector.reciprocal_approx_fast(ss[0:64, QCW:2 * QCW],
                                                 otp[1][0:64, :])
                nc.scalar.copy(rs[64:128, :], ss[0:64, QCW:2 * QCW])
                nc.vector.tensor_mul(oa[64:128, :], otp[1][64:128, :], rs[64:128, :])
                oall[qc][pr] = oa

                # ---- phase D: output projection at query-chunk boundaries ----
                if pr == 3:
                    emit_yproj(qc)


def build_nc():
    nc = bacc.Bacc("TRN2", target_bir_lowering=False, debug=False,
                   num_devices=NCORES)
    xh = nc.dram_tensor("xh", [C, ROWS], F16, kind="ExternalInput")
    wqkv = nc.dram_tensor("wqkv", [C, 3 * C], F16, kind="ExternalInput")
    wout = nc.dram_tensor("wout", [C, C], F16, kind="ExternalInput")
    bout = nc.dram_tensor("bout", [1, C], F16, kind="ExternalInput")
    mask = nc.dram_tensor("mask", [128, NQC * ESW], BF16, kind="ExternalInput")
    y = nc.dram_tensor("y", [CHUNK, C], F32, kind="ExternalOutput")
    with tile.TileContext(nc) as tc:
        build_attention_body(tc, y[:], xh[:], wqkv[:], wout[:], bout[:], mask[:])
    nc.compile()
    return nc


def _make_mask(qs: int) -> np.ndarray:
    """Band+edge mask for the core whose first query is global index qs.

    mask[p, 1536*qc + cum] = 1 iff key (local row 512*qc + 128*kt + p, i.e.
    global qs + 512*qc + 128*kt + p - 128) is within WIN of query
    (global qs + 512*qc + q) and inside [0, T).
    """
    import ml_dtypes
    m = np.zeros((128, NQC * ESW), dtype=ml_dtypes.bfloat16)
    p = np.arange(128)[:, None]
    for qc in range(NQC):
        for kt, c0, c1, q0, _, _ in _S_PIECES:
            q = (q0 + np.arange(c1 - c0))[None, :]
            band = np.abs(128 * kt + p - q - 128) <= WIN
            keyg = qs + 512 * qc + 128 * kt + p - 128
            inseq = (keyg >= 0) & (keyg < T)
            m[:, ESW * qc + c0:ESW * qc + c1] = (band & inseq).astype(ml_dtypes.bfloat16)
    return m


def make_in_maps(x, W_qkv, W_out, b_out):
    """Shard the full inputs into 8 per-core input maps."""
    x = np.asarray(x, dtype=np.float32)
    wqkv = np.asarray(W_qkv, dtype=np.float32).copy()
    wqkv[:, :C] *= SCALE  # fold hd^-0.5 into the q projection
    # permute the v-block columns so the projection writes v in the fused
    # [V_even | V_odd] layout the AV matmuls consume
    wv = wqkv[:, 2 * C:3 * C].reshape(C, H, HD)
    wqkv[:, 2 * C:3 * C] = wv[:, [0, 2, 4, 6, 1, 3, 5, 7]].reshape(C, C)
    wqkv = wqkv.astype(np.float16)
    wout = np.asarray(W_out, dtype=np.float32).astype(np.float16)
    bo = np.asarray(b_out, dtype=np.float32).astype(np.float16).reshape(1, C)
    in_maps = []
    for core in range(NCORES):
        b, ch = divmod(core, 4)
        qs = CHUNK * ch
        xhalo = np.zeros((ROWS, C), dtype=np.float16)
        g0, g1 = qs - WIN, qs + CHUNK + WIN
        s0, s1 = max(g0, 0), min(g1, T)
        xhalo[s0 - g0:s1 - g0, :] = x[b, s0:s1, :].astype(np.float16)
        xhalo = np.ascontiguousarray(xhalo.T)
        in_maps.append(dict(xh=xhalo, wqkv=wqkv, wout=wout, bout=bo,
                            mask=_make_mask(qs)))
    return in_maps


_CACHED_NC = None


def run_sharded(x, W_qkv, W_out, b_out, **run_kwargs):
    """Build (cached), run on 8 cores, gather. Returns (y_full, BassKernelResults)."""
    global _CACHED_NC
    if _CACHED_NC is None:
        _CACHED_NC = build_nc()
    in_maps = make_in_maps(x, W_qkv, W_out, b_out)
    res = run_bass_kernel_spmd(_CACHED_NC, in_maps, core_ids=list(range(NCORES)),
                               **run_kwargs)
    y_full = np.empty((B, T, C), dtype=np.float32)
    for core in range(NCORES):
        b, ch = divmod(core, 4)
        y_full[b, CHUNK * ch:CHUNK * (ch + 1), :] = res.results[core]["y"]
    return y_full, res


def kernel(x, W_qkv, W_out, b_out):
    y, _ = run_sharded(x, W_qkv, W_out, b_out)
    return y
